# revision 3
# baseline (speedup 1.0000x reference)
"""Trainium2 Bass kernel for an NIC-decoder style module:
embedding gather + dropout -> LSTM over T=41 steps -> dropout -> vocab projection.

Strategy: all 8 cores replicate the (sequential) LSTM; the large output
projection is tensor-parallel over the vocab dimension (1280 padded cols/core).
All matmuls run in float32r (TF32) on the PE at full rate; elementwise in fp32.
"""

import numpy as np

# problem shapes (hardcoded per contract)
B = 64
T = 41          # sequence length of the LSTM (1 image step + 40 word steps)
E = 512
H = 512
G = 2048        # 4*H gate width
V = 10000
VP = 10240      # padded vocab
NCORES = 8
VS = VP // NCORES   # 1280 vocab cols per core
NTOK = B * T        # 2624 tokens, t-major order: n = t*64 + b
NGRP = 20           # gather groups of 128 word tokens (2560 = 20*128)
KC = 4              # contraction chunks of 128 (E = H = 512)

_CACHE = {}


def _host_masks():
    """Reproduce the reference's fixed dropout masks (jax PRNG, key 42)."""
    import jax
    import jax.numpy as jnp

    with jax.default_device(jax.devices("cpu")[0]):
        dk = jax.random.split(jax.random.key(42), 3)
        keep = 0.5
        wm = jax.random.bernoulli(dk[0], keep, (B, 1, E)).astype(jnp.float32) / keep
        rm = jax.random.bernoulli(dk[1], keep, (B, H)).astype(jnp.float32) / keep
        fm = jax.random.bernoulli(dk[2], keep, (B, T, H)).astype(jnp.float32) / keep
        return (np.asarray(wm)[:, 0, :], np.asarray(rm), np.asarray(fm))


def _to_pkx(a, ncols):
    """[512, ncols] -> [128, 4, ncols] with partition p, k-chunk axis."""
    return np.ascontiguousarray(a.reshape(KC, 128, ncols).transpose(1, 0, 2))


def build_nc():
    import concourse.bass as bass
    import concourse.mybir as mybir
    import concourse.tile as tile
    from concourse import bacc
    from concourse.masks import make_identity

    fp32 = mybir.dt.float32
    fp32r = mybir.dt.float32r
    i32 = mybir.dt.int32
    ADD = mybir.AluOpType.add
    MUL = mybir.AluOpType.mult
    SIG = mybir.ActivationFunctionType.Sigmoid
    TANH = mybir.ActivationFunctionType.Tanh

    nc = bacc.Bacc("TRN2", target_bir_lowering=False, debug=False,
                   num_devices=NCORES)

    # ---- DRAM I/O ----
    emb_d = nc.dram_tensor("emb", [V, E], fp32, kind="ExternalInput")
    img_d = nc.dram_tensor("img", [B, E], fp32, kind="ExternalInput")
    idx_d = nc.dram_tensor("idx", [128, NGRP], i32, kind="ExternalInput")
    w_d = nc.dram_tensor("w", [128, KC, G], fp32r, kind="ExternalInput")
    u_d = nc.dram_tensor("u", [128, KC, G], fp32r, kind="ExternalInput")
    wo_d = nc.dram_tensor("wo", [128, KC, VS], fp32r, kind="ExternalInput")
    bb_d = nc.dram_tensor("bb", [128, G], fp32, kind="ExternalInput")
    bob_d = nc.dram_tensor("bob", [128, VS], fp32, kind="ExternalInput")
    wmt_d = nc.dram_tensor("wmt", [128, KC, B], fp32, kind="ExternalInput")
    rmt_d = nc.dram_tensor("rmt", [128, KC, B], fp32, kind="ExternalInput")
    fmt_d = nc.dram_tensor("fmt", [T, 128, KC, B], fp32, kind="ExternalInput")
    out_d = nc.dram_tensor("out", [NTOK, VS], fp32, kind="ExternalOutput")
    xz_d = nc.dram_tensor("xz_scratch", [NTOK, G], fp32)

    NCHUNK = (NTOK + 127) // 128  # 21 token chunks (last has 64 rows)

    def chunk_rows(m):
        return 128 if m < NCHUNK - 1 else NTOK - 128 * (NCHUNK - 1)

    with tile.TileContext(nc) as tc:
        with tc.tile_pool(name="shared", bufs=1) as shared:
            # constants resident for the whole kernel
            u_s = shared.tile([128, KC, G], fp32r)
            wo_s = shared.tile([128, KC, VS], fp32r)
            bob_s = shared.tile([128, VS], fp32)
            rmt_s = shared.tile([128, KC, B], fp32)
            ident = shared.tile([128, 128], fp32)
            nc.sync.dma_start(out=u_s, in_=u_d[:])
            nc.sync.dma_start(out=wo_s, in_=wo_d[:])
            nc.sync.dma_start(out=bob_s, in_=bob_d[:])
            nc.sync.dma_start(out=rmt_s, in_=rmt_d[:])
            make_identity(nc, ident)

            # ---------------- phase 1: gather + transpose + input GEMM ------
            with tc.tile_pool(name="ph1", bufs=1) as ph1:
                w_s = ph1.tile([128, KC, G], fp32r)
                bb_s = ph1.tile([128, G], fp32)
                wmt_s = ph1.tile([128, KC, B], fp32)
                idx_s = ph1.tile([128, NGRP], i32)
                img_s = ph1.tile([B, E], fp32)
                xseqT = ph1.tile([128, KC, NTOK], fp32r)
                nc.sync.dma_start(out=w_s, in_=w_d[:])
                nc.sync.dma_start(out=bb_s, in_=bb_d[:])
                nc.sync.dma_start(out=wmt_s, in_=wmt_d[:])
                nc.sync.dma_start(out=idx_s, in_=idx_d[:])
                nc.sync.dma_start(out=img_s, in_=img_d[:])

                # image step -> x_seqT[:, :, 0:64]
                with tc.tile_pool(name="tp_ps", bufs=4, space="PSUM") as tps:
                    for j in range(KC):
                        pt = tps.tile([128, B], fp32)
                        nc.tensor.transpose(
                            out=pt, in_=img_s[:, 128 * j:128 * (j + 1)],
                            identity=ident[:B, :B])
                        nc.vector.tensor_copy(xseqT[:, j, 0:B], pt)

                    # word tokens: gather 128 rows/group, transpose, word-mask
                    with tc.tile_pool(name="stage", bufs=4) as stg:
                        for g in range(NGRP):
                            st = stg.tile([128, E], fp32)
                            nc.gpsimd.indirect_dma_start(
                                out=st, out_offset=None, in_=emb_d[:],
                                in_offset=bass.IndirectOffsetOnAxis(
                                    ap=idx_s[:, g:g + 1], axis=0))
                            base = B + 128 * g
                            for j in range(KC):
                                pt = tps.tile([128, 128], fp32)
                                nc.tensor.transpose(
                                    out=pt, in_=st[:, 128 * j:128 * (j + 1)],
                                    identity=ident)
                                dst = xseqT[:, j, base:base + 128].rearrange(
                                    "p (u b) -> p u b", u=2)
                                src = pt.rearrange("p (u b) -> p u b", u=2)
                                msk = bass.AP(
                                    tensor=wmt_s.tensor,
                                    offset=wmt_s[:, j, :].offset,
                                    ap=[list(wmt_s.ap[0]), [0, 2],
                                        list(wmt_s[:, j, :].ap[-1])])
                                nc.vector.tensor_tensor(
                                    out=dst, in0=src, in1=msk, op=MUL)

                # input GEMM: xz = x_seq @ W + b  (token chunks of 128)
                with tc.tile_pool(name="xz_ps", bufs=2, space="PSUM") as xzp, \
                        tc.tile_pool(name="xz_sb", bufs=3) as xzb:
                    for m in range(NCHUNK):
                        rows = chunk_rows(m)
                        ps = xzp.tile([128, G], fp32)
                        for n in range(4):
                            for k in range(KC):
                                nc.tensor.matmul(
                                    out=ps[:rows, 512 * n:512 * (n + 1)],
                                    lhsT=xseqT[:, k, 128 * m:128 * m + rows]
                                        ,
                                    rhs=w_s[:, k, 512 * n:512 * (n + 1)]
                                        ,
                                    start=(k == 0), stop=(k == KC - 1))
                        sb = xzb.tile([128, G], fp32)
                        nc.vector.tensor_tensor(
                            out=sb[:rows], in0=ps[:rows], in1=bb_s[:rows],
                            op=ADD)
                        nc.sync.dma_start(
                            out=xz_d[128 * m:128 * m + rows, :], in_=sb[:rows])

            # ---------------- phase 2: LSTM recurrence + projection ---------
            with tc.tile_pool(name="ph2", bufs=1) as ph2, \
                    tc.tile_pool(name="hmp", bufs=1) as hmp, \
                    tc.tile_pool(name="xzt", bufs=3) as xzt_pool, \
                    tc.tile_pool(name="fmt", bufs=3) as fmt_pool, \
                    tc.tile_pool(name="gat", bufs=2) as gat, \
                    tc.tile_pool(name="st8", bufs=2) as st8, \
                    tc.tile_pool(name="ob", bufs=3) as obp, \
                    tc.tile_pool(name="z_ps", bufs=1, space="PSUM") as zps, \
                    tc.tile_pool(name="s_ps", bufs=4, space="PSUM") as sps:
                hmT = hmp.tile([128, KC, NTOK], fp32r)
                c_prev = None
                rhT_prev = None

                def project(m):
                    rows = chunk_rows(m)
                    for vc0, vlen in ((0, 512), (512, 512), (1024, 256)):
                        pp = sps.tile([128, 512], fp32, tag="proj")
                        for k in range(KC):
                            nc.tensor.matmul(
                                out=pp[:rows, :vlen],
                                lhsT=hmT[:, k, 128 * m:128 * m + rows]
                                    ,
                                rhs=wo_s[:, k, vc0:vc0 + vlen],
                                start=(k == 0), stop=(k == KC - 1))
                        ob = obp.tile([128, 512], fp32)
                        nc.vector.tensor_tensor(
                            out=ob[:rows, :vlen], in0=pp[:rows, :vlen],
                            in1=bob_s[:rows, vc0:vc0 + vlen], op=ADD)
                        nc.sync.dma_start(
                            out=out_d[128 * m:128 * m + rows, vc0:vc0 + vlen],
                            in_=ob[:rows, :vlen])

                for t in range(T):
                    # prefetch xz_t and final-mask_t
                    xz_t = xzt_pool.tile([B, G], fp32)
                    nc.scalar.dma_start(
                        out=xz_t, in_=xz_d[B * t:B * (t + 1), :])
                    fmt_t = fmt_pool.tile([128, KC, B], fp32)
                    nc.scalar.dma_start(out=fmt_t, in_=fmt_d[t])

                    if t > 0:
                        zp = zps.tile([B, G], fp32)
                        for n in range(4):
                            for k in range(KC):
                                nc.tensor.matmul(
                                    out=zp[:, 512 * n:512 * (n + 1)],
                                    lhsT=rhT_prev[:, k, :],
                                    rhs=u_s[:, k, 512 * n:512 * (n + 1)]
                                        ,
                                    start=(k == 0), stop=(k == KC - 1))

                    # gates (order i, f, g, o)
                    zf = []
                    for n in range(4):
                        if t > 0:
                            zn = gat.tile([B, 512], fp32, tag="zf")
                            nc.vector.tensor_tensor(
                                out=zn, in0=zp[:, 512 * n:512 * (n + 1)],
                                in1=xz_t[:, 512 * n:512 * (n + 1)], op=ADD)
                        else:
                            zn = xz_t[:, 512 * n:512 * (n + 1)]
                        act = gat.tile([B, 512], fp32, tag=f"g{n}")
                        nc.scalar.activation(
                            out=act, in_=zn, func=(TANH if n == 2 else SIG))
                        zf.append(act)
                    g_i, g_f, g_g, g_o = zf

                    c_t = st8.tile([B, H], fp32, tag="c")
                    if t > 0:
                        m1 = gat.tile([B, H], fp32, tag="m1")
                        nc.gpsimd.tensor_tensor(out=m1, in0=g_i, in1=g_g,
                                                op=MUL)
                        m2 = gat.tile([B, H], fp32, tag="m2")
                        nc.gpsimd.tensor_tensor(out=m2, in0=g_f, in1=c_prev,
                                                op=MUL)
                        nc.gpsimd.tensor_tensor(out=c_t, in0=m1, in1=m2,
                                                op=ADD)
                    else:
                        nc.gpsimd.tensor_tensor(out=c_t, in0=g_i, in1=g_g,
                                                op=MUL)
                    tc_t = gat.tile([B, H], fp32, tag="tc")
                    nc.scalar.activation(out=tc_t, in_=c_t, func=TANH)
                    h_t = st8.tile([B, H], fp32, tag="h")
                    nc.vector.tensor_tensor(out=h_t, in0=g_o, in1=tc_t, op=MUL)

                    # transpose h, apply recurrent + final dropout masks
                    rhT = st8.tile([128, KC, B], fp32r, tag="rhT")
                    for j in range(KC):
                        pt = sps.tile([128, 512], fp32, tag="proj")
                        nc.tensor.transpose(
                            out=pt[:, :B], in_=h_t[:, 128 * j:128 * (j + 1)],
                            identity=ident[:B, :B])
                        nc.vector.tensor_tensor(
                            out=rhT[:, j, :], in0=pt[:, :B],
                            in1=rmt_s[:, j, :], op=MUL)
                        nc.vector.tensor_tensor(
                            out=hmT[:, j, B * t:B * (t + 1)], in0=pt[:, :B],
                            in1=fmt_t[:, j, :], op=MUL)
                    c_prev, rhT_prev = c_t, rhT

                    # inline vocab projection once both steps of a chunk done
                    if t % 2 == 1:
                        project((t - 1) // 2)
                project(NCHUNK - 1)

    nc.compile()
    return nc


def _prep_inputs(pos_embs, images_emb, targets, emb_table, W, U, b, Wo, bo):
    wm, rm, fm = _host_masks()
    targets = np.asarray(targets).astype(np.int32)
    idx_tm = np.ascontiguousarray(targets[:, :40].T).reshape(-1)  # [2560]
    idx_h = np.ascontiguousarray(idx_tm.reshape(NGRP, 128).T)    # [128, 20]

    Wo_p = np.zeros((H, VP), np.float32)
    Wo_p[:, :V] = Wo
    bo_p = np.zeros((VP,), np.float32)
    bo_p[:V] = bo

    fmt = np.ascontiguousarray(
        fm.transpose(1, 0, 2)            # [T, B, H]
        .transpose(0, 2, 1)              # [T, H, B]
        .reshape(T, KC, 128, B)
        .transpose(0, 2, 1, 3))          # [T, 128, KC, B]

    common = {
        "emb": np.ascontiguousarray(emb_table, dtype=np.float32),
        "img": np.ascontiguousarray(images_emb, dtype=np.float32),
        "idx": idx_h,
        "w": _to_pkx(np.asarray(W, np.float32), G),
        "u": _to_pkx(np.asarray(U, np.float32), G),
        "bb": np.ascontiguousarray(
            np.broadcast_to(np.asarray(b, np.float32), (128, G))),
        "wmt": _to_pkx(np.ascontiguousarray(wm.T), B),
        "rmt": _to_pkx(np.ascontiguousarray(rm.T), B),
        "fmt": fmt,
    }
    in_maps = []
    for c in range(NCORES):
        m = dict(common)
        m["wo"] = _to_pkx(
            np.ascontiguousarray(Wo_p[:, VS * c:VS * (c + 1)]), VS)
        m["bob"] = np.ascontiguousarray(
            np.broadcast_to(bo_p[VS * c:VS * (c + 1)], (128, VS)))
        in_maps.append(m)
    return in_maps


def kernel(pos_embs, images_emb, targets, emb_table, W, U, b, Wo, bo):
    from concourse.bass_utils import run_bass_kernel_spmd

    if "nc" not in _CACHE:
        _CACHE["nc"] = build_nc()
    nc = _CACHE["nc"]
    in_maps = _prep_inputs(pos_embs, images_emb, targets, emb_table,
                           W, U, b, Wo, bo)
    res = run_bass_kernel_spmd(nc, in_maps, list(range(NCORES)))
    parts = [r["out"].reshape(T, B, VS).transpose(1, 0, 2)
             for r in res.results]
    return np.ascontiguousarray(
        np.concatenate(parts, axis=2)[:, :, :V]).astype(np.float32)
